# revision 3
# baseline (speedup 1.0000x reference)
"""Additive-attention (ContentAttender) Bass kernel for 8 TRN2 NeuronCores.

Problem: B=4, NQ=512, NK=512, D=128, H=32
  kh = keys @ Wk; qh = queries @ Wq
  logits[b,q,k] = w2 . tanh(qh[b,q] + kh[b,k] + b1) + b2
  out = softmax_k(logits) @ keys

Sharding: data-parallel over (batch x query-half) -> 8 cores, each core
handles one batch's 256 queries vs all 512 keys. No collectives.

Per-core layout trick: queries are packed 4-per-32-partition-group so the
(q,k,h) tanh tensor is computed as 64 DVE broadcast-adds (khT4 + per-group
query bias) + big-chunk ScalarE tanh, and the h-contraction with w2 runs on
the TensorEngine via a host-built block-diagonal weight matrix.

softmax skips max-subtraction: |logits| <= sum|w2| + |b2| ~ 3, safe in fp32.
b2 is dropped entirely (softmax is shift-invariant).
"""

import numpy as np
import ml_dtypes

import concourse.bass as bass  # noqa: F401  (bass must be importable)
import concourse.mybir as mybir
import concourse.tile as tile
from concourse import bacc
from concourse.bass_utils import run_bass_kernel_spmd

F32 = mybir.dt.float32
BF16 = mybir.dt.bfloat16
AF = mybir.ActivationFunctionType

B, NQ, NK, D, H = 4, 512, 512, 128, 32
NQC = NQ // 2          # queries per core = 256
NG = NQC // 4          # query groups per core = 64
CHUNK = 16             # groups per tanh chunk
NCHUNK = NG // CHUNK   # 4

_CACHED_NC = None


def _build_nc():
    nc = bacc.Bacc("TRN2", target_bir_lowering=False, debug=False)

    keysT = nc.declare_dram_parameter("keysT", [128, NK], F32, isOutput=False)
    kctx = nc.declare_dram_parameter("kctx", [128, NK], BF16, isOutput=False)
    queriesT = nc.declare_dram_parameter("queriesT", [128, NQC], F32, isOutput=False)
    Wk = nc.declare_dram_parameter("Wk", [D, H], F32, isOutput=False)
    Wq = nc.declare_dram_parameter("Wq", [D, H], F32, isOutput=False)
    b1r = nc.declare_dram_parameter("b1r", [1, H], F32, isOutput=False)
    W2D = nc.declare_dram_parameter("W2D", [128, 8 * H], BF16, isOutput=False)
    ident = nc.declare_dram_parameter("ident", [128, 128], BF16, isOutput=False)
    out = nc.declare_dram_parameter("out", [NQC, D], F32, isOutput=True)

    with tile.TileContext(nc) as tc:
        import contextlib

        with contextlib.ExitStack() as ctx:
            cpool = ctx.enter_context(tc.tile_pool(name="consts", bufs=1))
            spool = ctx.enter_context(tc.tile_pool(name="schunk", bufs=2))
            epool = ctx.enter_context(tc.tile_pool(name="softmax", bufs=2))
            ps_setup = ctx.enter_context(
                tc.tile_pool(name="ps_setup", bufs=1, space="PSUM")
            )
            ps_logits = ctx.enter_context(
                tc.tile_pool(name="ps_logits", bufs=2, space="PSUM")
            )
            ps_tr = ctx.enter_context(tc.tile_pool(name="ps_tr", bufs=2, space="PSUM"))
            ps_ctx = ctx.enter_context(
                tc.tile_pool(name="ps_ctx", bufs=2, space="PSUM")
            )

            # ---- load inputs ----
            kT = cpool.tile([128, NK], F32, tag="kT")
            nc.sync.dma_start(kT[:], keysT[:])
            kctx_sb = cpool.tile([128, NK], BF16, tag="kctx")
            nc.sync.dma_start(kctx_sb[:], kctx[:])
            qT = cpool.tile([128, NQC], F32, tag="qT")
            nc.sync.dma_start(qT[:], queriesT[:])
            Wk_sb = cpool.tile([D, H], F32, tag="Wk")
            nc.sync.dma_start(Wk_sb[:], Wk[:])
            Wq_sb = cpool.tile([D, H], F32, tag="Wq")
            nc.sync.dma_start(Wq_sb[:], Wq[:])
            b1_sb = cpool.tile([1, H], F32, tag="b1")
            nc.sync.dma_start(b1_sb[:], b1r[:])
            W2D_sb = cpool.tile([128, 8 * H], BF16, tag="W2D")
            nc.sync.dma_start(W2D_sb[:], W2D[:])
            id_sb = cpool.tile([128, 128], BF16, tag="ident")
            nc.sync.dma_start(id_sb[:], ident[:])

            ones_sb = cpool.tile([1, NK], F32, tag="ones")
            nc.gpsimd.memset(ones_sb[:], 1.0)

            # ---- khbT = Wk.T @ keysT + b1 (32, 512); qhT = Wq.T @ queriesT ----
            khbT_ps = ps_setup.tile([H, NK], F32, tag="setup")
            nc.tensor.matmul(khbT_ps[:], Wk_sb[:], kT[:], start=True, stop=False)
            nc.tensor.matmul(
                khbT_ps[:], b1_sb[:], ones_sb[:, :NK], start=False, stop=True
            )
            khbT_sb = cpool.tile([H, NK], F32, tag="khbT")
            nc.vector.tensor_copy(khbT_sb[:], khbT_ps[:])

            qhT_ps = ps_setup.tile([H, NQC], F32, tag="setup")
            nc.tensor.matmul(qhT_ps[:], Wq_sb[:], qT[:], start=True, stop=True)
            qhT_sb = cpool.tile([H, NQC], F32, tag="qhT")
            nc.vector.tensor_copy(qhT_sb[:], qhT_ps[:])

            # ---- khT4: khbT replicated on 4 partition groups (128, 512) ----
            khT4 = cpool.tile([128, NK], F32, tag="khT4")
            for j in range(4):
                nc.sync.dma_start(khT4[32 * j : 32 * j + 32, :], khbT_sb[:])

            # ---- QB4[(j,h), g] = qhT[h, 64j + g]  (128, 64) ----
            QB4 = cpool.tile([128, NG], F32, tag="QB4")
            for j in range(4):
                nc.sync.dma_start(
                    QB4[32 * j : 32 * j + 32, :], qhT_sb[:, NG * j : NG * (j + 1)]
                )

            # ---- main: chunks of CHUNK groups ----
            logits_ps = [None, None]

            def emit_chunk(c):
                S = spool.tile([128, CHUNK * NK], F32, tag="S")
                Sb = spool.tile([128, CHUNK * NK], BF16, tag="Sb")
                for gl in range(CHUNK):
                    g = CHUNK * c + gl
                    nc.vector.tensor_scalar_add(
                        S[:, NK * gl : NK * (gl + 1)], khT4[:], QB4[:, g : g + 1]
                    )
                nc.scalar.activation(Sb[:], S[:], AF.Tanh)
                for gl in range(CHUNK):
                    g = CHUNK * c + gl
                    beta = g // 32
                    s = (g // 8) % 4
                    g8 = g % 8
                    if logits_ps[beta] is None:
                        logits_ps[beta] = ps_logits.tile(
                            [128, NK], F32, tag="logits", name=f"logits{beta}"
                        )
                    nc.tensor.matmul(
                        logits_ps[beta][32 * s : 32 * s + 32, :],
                        W2D_sb[:, 32 * g8 : 32 * g8 + 32],
                        Sb[:, NK * gl : NK * (gl + 1)],
                        start=(g8 == 0),
                        stop=(g8 == 7),
                        tile_position=(0, 32 * s),
                    )

            def emit_tail(beta):
                E = epool.tile([128, NK], BF16, tag="E")
                nc.scalar.activation(E[:], logits_ps[beta][:], AF.Exp)
                rs = epool.tile([128, 1], F32, tag="rs")
                nc.vector.reduce_sum(rs[:], E[:], axis=mybir.AxisListType.X)
                rr = epool.tile([128, 1], F32, tag="rr")
                nc.vector.reciprocal(rr[:], rs[:])
                ET = epool.tile([128, NK], BF16, tag="ET")
                for t in range(4):
                    trp = ps_tr.tile([128, 128], BF16, tag="tr")
                    nc.tensor.transpose(
                        trp[:], E[:, 128 * t : 128 * (t + 1)], id_sb[:]
                    )
                    nc.vector.tensor_copy(ET[:, 128 * t : 128 * (t + 1)], trp[:])
                ctxp = ps_ctx.tile([128, D], F32, tag="ctx")
                for t in range(4):
                    nc.tensor.matmul(
                        ctxp[:],
                        ET[:, 128 * t : 128 * (t + 1)],
                        kctx_sb[:, 128 * t : 128 * (t + 1)],
                        start=(t == 0),
                        stop=(t == 3),
                    )
                ctx_sb = epool.tile([128, D], F32, tag="ctxs")
                nc.vector.tensor_scalar_mul(ctx_sb[:], ctxp[:], rr[:])
                nc.sync.dma_start(out[128 * beta : 128 * (beta + 1), :], ctx_sb[:])

            emit_chunk(0)
            emit_chunk(1)
            emit_chunk(2)
            emit_tail(0)
            emit_chunk(3)
            emit_tail(1)

    nc.compile()
    return nc


def _get_nc():
    global _CACHED_NC
    if _CACHED_NC is None:
        _CACHED_NC = _build_nc()
    return _CACHED_NC


def _build_w2d(w2):
    """(128, 256) block weights: slice g8 has column 4*g8+j = w2 on
    partitions [32j, 32j+32), zeros elsewhere."""
    w2d = np.zeros((128, 8 * H), np.float32)
    for g8 in range(8):
        for j in range(4):
            w2d[32 * j : 32 * j + 32, 32 * g8 + 4 * g8 + j] = w2
    return w2d.astype(ml_dtypes.bfloat16)


def _qmap():
    """out row r -> local query index."""
    r = np.arange(NQC)
    beta = r // 128
    p = r % 128
    return 64 * (p % 4) + 32 * beta + 8 * (p // 32) + (p % 32) // 4


def _in_maps(keys, queries, Wk, Wq, b1, w2):
    keys = np.asarray(keys, np.float32)
    queries = np.asarray(queries, np.float32)
    w2d = _build_w2d(np.asarray(w2, np.float32))
    ident = np.eye(128, dtype=ml_dtypes.bfloat16)
    Wk = np.ascontiguousarray(np.asarray(Wk, np.float32))
    Wq = np.ascontiguousarray(np.asarray(Wq, np.float32))
    b1r = np.ascontiguousarray(np.asarray(b1, np.float32).reshape(1, H))
    maps = []
    for c in range(8):
        b, half = divmod(c, 2)
        kb = keys[b]  # (512, 128)
        maps.append(
            {
                "keysT": np.ascontiguousarray(kb.T),
                "kctx": np.ascontiguousarray(
                    kb.reshape(4, 128, 128).transpose(1, 0, 2).reshape(128, 512)
                ).astype(ml_dtypes.bfloat16),
                "queriesT": np.ascontiguousarray(
                    queries[b, NQC * half : NQC * (half + 1)].T
                ),
                "Wk": Wk,
                "Wq": Wq,
                "b1r": b1r,
                "W2D": w2d,
                "ident": ident,
            }
        )
    return maps


def _run(in_maps, trace=False):
    nc = _get_nc()
    return run_bass_kernel_spmd(nc, in_maps, core_ids=list(range(8)), trace=trace)


def kernel(keys, queries, Wk, Wq, b1, w2, b2):
    res = _run(_in_maps(keys, queries, Wk, Wq, b1, w2))
    qmap = _qmap()
    outv = np.empty((B, NQ, D), np.float32)
    for c in range(8):
        b, half = divmod(c, 2)
        outv[b, NQC * half + qmap] = res.results[c]["out"]
    return outv
